# revision 11
# baseline (speedup 1.0000x reference)
"""Bidirectional Mamba block on 8 Trainium2 NeuronCores.

Sharding: core i handles (dir = i//4, sample = i%4). SPMD: identical program,
per-core data (bwd cores get L-reversed input + their direction's weights).
The selective scan runs as 24 row-tiles of 128 partitions (rows = (d, n)
pairs, d-major) using the DVE tensor_tensor_scan instruction along the free
(sequence) dimension; per-tile (d,n)-expansion of delta / delta*u uses small
K=32 selector matmuls, dA = exp(delta*A) is one ACT op with a per-partition
scale, and the n-reduction back to d-rows is a K=128 selector matmul
accumulated in PSUM. out_proj and the fuse 1x1 conv are folded into a single
matmul (G = out_W @ fuse_W_half.T, host-precomputed). Cross-direction
fuse-sum + train-mode batchnorm run on device via collectives; the bwd cores'
sequence un-reversal is a data-driven mask blend (no control flow).
"""
import numpy as np
from contextlib import ExitStack

import concourse.bacc as bacc
import concourse.bass as bass
import concourse.tile as tile
import concourse.mybir as mybir
from concourse.bass_utils import run_bass_kernel_spmd

dt = mybir.dt
F32 = dt.float32
BF16 = dt.bfloat16
AF = mybir.ActivationFunctionType
ALU = mybir.AluOpType

# problem constants (hardcoded; kernel.py must be self-contained)
BSZ, DIM, HH, WW = 4, 96, 64, 64
L = HH * WW              # 4096
NST = 16                 # d_state
DI = 2 * DIM             # 192
DTR = DIM // 16          # 6
KC = 4                   # d_conv
NCORES = 8
TCH = 512                # psum chunk (fp32 bank)
NCH = L // TCH           # 8
NT = DI * NST // 128     # 24 row tiles; tile t = d in [8t, 8t+8), all 16 n
HLF = 2                  # compact halves of 96 channels
BN_CNT = 2 * BSZ * L     # stats AllReduce double-counts each sample
# xproj output layout (partition-aligned for PE reads): B@0:16, C@32:48,
# dt@64:70 within a 70-row tensor
XP_M = 70
XB, XC_, XD = 0, 32, 64


def build_program():
    nc = bacc.Bacc("TRN2", target_bir_lowering=False, debug=False,
                   num_devices=NCORES)

    def din(name, shape):
        return nc.dram_tensor(name, list(shape), F32, kind="ExternalInput").ap()

    xT = din("xT", (DIM, L))
    inW = din("inW", (DIM, 2 * DI))
    convw = din("convw", (DIM, 2 * KC))
    convb = din("convb", (DIM, HLF))
    xpWp = din("xpWp", (DI, XP_M))          # permuted/padded xproj weights
    dtW = din("dtW", (DTR, DI))
    dtb = din("dtb", (DIM, HLF))
    Dv = din("Dv", (DIM, HLF))
    Asc = din("Asc", (128, NT))
    E8B = din("E8B", (128, 512))            # 4 col-blocks of [32,128] selectors
    E16D = din("E16D", (48, 128))           # rows 0:16 and 32:48 both E16
    SELB = din("SELB", (128, 128))          # 4 col-blocks of [128,32] selectors
    Gw = din("Gw", (DI, DIM))
    m0 = din("m0", (DIM, 1))
    m1 = din("m1", (DIM, 1))
    fuseb = din("fuseb", (DIM, 1))
    bng = din("bng", (DIM, 1))
    bnb = din("bnb", (DIM, 1))

    out = nc.dram_tensor("out", [DIM, L], F32, kind="ExternalOutput").ap()

    with tile.TileContext(nc) as tc, ExitStack() as ctx:
        cst = ctx.enter_context(tc.tile_pool(name="cst", bufs=1))

        def load(ap_dram, shape, tag, dtype=F32):
            t = cst.tile(list(shape), dtype, name=tag, tag=tag)
            nc.sync.dma_start(t[:], ap_dram)
            return t

        inW_s = load(inW, (DIM, 2 * DI), 'inW')
        convw_s = load(convw, (DIM, 2 * KC), 'convw')
        convb_s = load(convb, (DIM, HLF), 'convb')
        xpW_s = [load(xpWp[96 * h:96 * (h + 1), :], (96, XP_M), f'xpW{h}')
                 for h in range(HLF)]
        # dtW placed at partitions 64:70 to match its rhs (dbc[64:70])
        dtW_s = cst.tile([XP_M, DI], F32, name='dtW_s', tag='dtW_s')
        nc.sync.dma_start(dtW_s[XD:XD + DTR, :], dtW)
        dtb_s = load(dtb, (DIM, HLF), 'dtb')
        Dv_s = load(Dv, (DIM, HLF), 'Dv')
        Asc_s = load(Asc, (128, NT), 'Asc')
        E8B_s = load(E8B, (128, 512), 'E8B')
        E16_s = load(E16D, (48, 128), 'E16')
        SELB_s = load(SELB, (128, 128), 'SELB')
        G_s = [load(Gw[96 * h:96 * (h + 1), :], (96, DIM), f'G{h}')
               for h in range(HLF)]
        m0_s = load(m0, (DIM, 1), 'm0')
        m1_s = load(m1, (DIM, 1), 'm1')
        fuseb_s = load(fuseb, (DIM, 1), 'fuseb')
        bng_s = load(bng, (DIM, 1), 'bng')
        bnb_s = load(bnb, (DIM, 1), 'bnb')

        big = ctx.enter_context(tc.tile_pool(name="big", bufs=1))
        u = [big.tile([96, L], F32, name=f'u{h}', tag=f'u{h}')
             for h in range(HLF)]
        z = [big.tile([96, L], F32, name=f'z{h}', tag=f'z{h}')
             for h in range(HLF)]
        delta = [big.tile([96, L], F32, name=f'delta{h}', tag=f'delta{h}')
                 for h in range(HLF)]
        dbc = big.tile([XP_M, L], F32, name='dbc', tag='dbc')
        Brows = big.tile([128, L], BF16, name='Brows', tag='Brows')
        Crows = big.tile([128, L], BF16, name='Crows', tag='Crows')
        carry = big.tile([128, NT], F32, name='carry', tag='carry')

        dram = ctx.enter_context(tc.tile_pool(name="dram", bufs=1,
                                              space="DRAM"))
        zstage = dram.tile([DIM, L], F32, name='zstage', tag='zstage')
        zin = dram.tile([DIM, L], F32, name='zin', tag='zin')
        zred = dram.tile([DIM, L], F32, name='zred', tag='zred')
        stin = dram.tile([DIM, 2], F32, name='stin', tag='stin')
        stred = dram.tile([DIM, 2], F32, name='stred', tag='stred')

        nc.vector.memset(carry[:], 0.0)
        # ---- Phase A: in-proj, conv, silu ----
        with tc.tile_pool(name="phA", bufs=1) as phA, \
             tc.tile_pool(name="xcp", bufs=2) as xcp, \
             tc.tile_pool(name="psA", bufs=4, space="PSUM") as psA:
            u_pad = [phA.tile([96, L + KC - 1], F32, name=f'u_pad{h}',
                              tag=f'u_pad{h}') for h in range(HLF)]
            for h in range(HLF):
                nc.vector.memset(u_pad[h][:, 0:KC - 1], 0.0)
            for c in range(NCH):
                cs = c * TCH
                xc = xcp.tile([DIM, TCH], F32, name='xc', tag='xc')
                nc.sync.dma_start(xc[:], xT[:, cs:cs + TCH])
                for g in range(4):
                    pt = psA.tile([96, TCH], F32, name='pt', tag='pt')
                    nc.tensor.matmul(pt[:], inW_s[:, 96 * g:96 * (g + 1)],
                                     xc[:], start=True, stop=True)
                    if g < 2:
                        nc.scalar.copy(
                            u_pad[g][:, KC - 1 + cs:KC - 1 + cs + TCH], pt[:])
                    else:
                        nc.scalar.copy(z[g - 2][:, cs:cs + TCH], pt[:])
            CCH = 512
            with tc.tile_pool(name="cscr", bufs=2) as cscr:
                for h in range(HLF):
                    for j in range(L // CCH):
                        o = j * CCH
                        sA = cscr.tile([96, CCH], F32, name='sA', tag='sA')
                        sB = cscr.tile([96, CCH], F32, name='sB', tag='sB')
                        nc.vector.tensor_scalar(
                            sA[:], u_pad[h][:, o + 3:o + 3 + CCH],
                            convw_s[:, 4 * h + 3:4 * h + 4], None,
                            op0=ALU.mult)
                        nc.vector.scalar_tensor_tensor(
                            sB[:], u_pad[h][:, o + 2:o + 2 + CCH],
                            convw_s[:, 4 * h + 2:4 * h + 3], sA[:],
                            op0=ALU.mult, op1=ALU.add)
                        nc.vector.scalar_tensor_tensor(
                            sA[:], u_pad[h][:, o + 1:o + 1 + CCH],
                            convw_s[:, 4 * h + 1:4 * h + 2], sB[:],
                            op0=ALU.mult, op1=ALU.add)
                        nc.vector.scalar_tensor_tensor(
                            sB[:], u_pad[h][:, o:o + CCH],
                            convw_s[:, 4 * h:4 * h + 1], sA[:],
                            op0=ALU.mult, op1=ALU.add)
                        ug = cscr.tile([96, CCH], F32, name='ug',
                                       tag='ug', bufs=2)
                        nc.vector.tensor_scalar(ug[:], sB[:],
                                                convb_s[:, h:h + 1], None,
                                                op0=ALU.add)
                        sg = cscr.tile([96, CCH], F32, name='sg',
                                       tag='sg', bufs=2)
                        nc.scalar.activation(sg[:], ug[:], AF.Sigmoid)
                        nc.vector.tensor_tensor(u[h][:, o:o + CCH], ug[:],
                                                sg[:], op=ALU.mult)

        # ---- xproj, delta, B/C row replication ----
        with tc.tile_pool(name="psB", bufs=1, space="PSUM") as psB, \
             tc.tile_pool(name="etp", bufs=3) as etp:
            for c in range(NCH):
                cs = c * TCH
                pj = psB.tile([XP_M, TCH], F32, name='pj', tag='pj', bufs=2)
                nc.tensor.matmul(pj[:], xpW_s[0][:], u[0][:, cs:cs + TCH],
                                 start=True, stop=False)
                nc.tensor.matmul(pj[:], xpW_s[1][:], u[1][:, cs:cs + TCH],
                                 start=False, stop=True)
                nc.scalar.copy(dbc[:, cs:cs + TCH], pj[:])
                for h in range(HLF):
                    pd = psB.tile([96, TCH], F32, name='pd', tag='pd', bufs=2)
                    nc.tensor.matmul(pd[:], dtW_s[XD:XD + DTR,
                                                  96 * h:96 * (h + 1)],
                                     dbc[XD:XD + DTR, cs:cs + TCH],
                                     start=True, stop=True)
                    et = etp.tile([96, TCH], F32, name='et', tag='et')
                    nc.scalar.activation(et[:], pd[:], AF.Exp,
                                         bias=dtb_s[:, h:h + 1])
                    nc.scalar.activation(delta[h][:, cs:cs + TCH], et[:],
                                         AF.Ln, bias=1.0)
                pb = psB.tile([128, TCH], F32, name='pb', tag='pb', bufs=2)
                nc.tensor.matmul(pb[:], E16_s[0:NST, :],
                                 dbc[XB:XB + NST, cs:cs + TCH],
                                 start=True, stop=True)
                nc.scalar.copy(Brows[:, cs:cs + TCH], pb[:])
                pc2 = psB.tile([128, TCH], F32, name='pc2', tag='pb', bufs=2)
                nc.tensor.matmul(pc2[:], E16_s[XC_:XC_ + NST, :],
                                 dbc[XC_:XC_ + NST, cs:cs + TCH],
                                 start=True, stop=True)
                nc.scalar.copy(Crows[:, cs:cs + TCH], pc2[:])

        # ---- Phase B: expanded scan ----
        with tc.tile_pool(name="psD", bufs=1, space="PSUM") as psD, \
             tc.tile_pool(name="psP", bufs=1, space="PSUM") as psP, \
             tc.tile_pool(name="psY", bufs=1, space="PSUM") as psY, \
             tc.tile_pool(name="psZ", bufs=1, space="PSUM") as psZ, \
             tc.tile_pool(name="scp", bufs=1) as scp, \
             tc.tile_pool(name="tlp", bufs=1) as tlp:
            for c in range(NCH):
                cs = c * TCH
                pY = [psY.tile([96, TCH], F32, name=f'y{h}', tag=f'y{h}',
                               bufs=1) for h in range(HLF)]
                pc_ = [tlp.tile([96, TCH], F32, name=f'pc{h}', tag=f'pc{h}',
                                bufs=2) for h in range(HLF)]
                for h in range(HLF):
                    nc.vector.tensor_tensor(pc_[h][:], delta[h][:, cs:cs + TCH],
                                            u[h][:, cs:cs + TCH], op=ALU.mult)
                for t in range(NT):
                    h = t // 12
                    ra = 32 * ((t % 12) // 4)
                    v = t % 4
                    pD = psD.tile([128, TCH], F32, name='pD', tag='pD', bufs=2)
                    nc.tensor.matmul(pD[:],
                                     E8B_s[ra:ra + 32, 128 * v:128 * (v + 1)],
                                     delta[h][ra:ra + 32, cs:cs + TCH],
                                     start=True, stop=True)
                    dA = scp.tile([128, TCH], F32, name='dA', tag='dA', bufs=2)
                    nc.scalar.activation(dA[:], pD[:], AF.Exp,
                                         scale=Asc_s[:, t:t + 1])
                    pP = psP.tile([128, TCH], F32, name='pP', tag='pP', bufs=2)
                    nc.tensor.matmul(pP[:],
                                     E8B_s[ra:ra + 32, 128 * v:128 * (v + 1)],
                                     pc_[h][ra:ra + 32, :],
                                     start=True, stop=True)
                    dBu = scp.tile([128, TCH], F32, name='dBu', tag='dBu',
                                   bufs=2)
                    nc.vector.tensor_tensor(dBu[:], pP[:],
                                            Brows[:, cs:cs + TCH],
                                            op=ALU.mult)
                    hs = scp.tile([128, TCH], F32, name='hs', tag='hs', bufs=2)
                    nc.vector.tensor_tensor_scan(hs[:], dA[:], dBu[:],
                                                 carry[:, t:t + 1],
                                                 op0=ALU.mult, op1=ALU.add)
                    nc.gpsimd.tensor_copy(carry[:, t:t + 1],
                                          hs[:, TCH - 1:TCH])
                    hC = scp.tile([128, TCH], F32, name='hC', tag='hC', bufs=2)
                    nc.gpsimd.tensor_tensor(hC[:], hs[:],
                                            Crows[:, cs:cs + TCH],
                                            op=ALU.mult)
                    nc.tensor.matmul(pY[h][ra:ra + 32, :],
                                     SELB_s[:, 32 * v:32 * (v + 1)], hC[:],
                                     start=(v == 0), stop=(v == 3),
                                     skip_group_check=True)
                pZ = psZ.tile([96, TCH], F32, name='pZ', tag='pZ', bufs=2)
                for h in range(HLF):
                    yf = tlp.tile([96, TCH], F32, name='yf', tag='yf', bufs=2)
                    nc.vector.scalar_tensor_tensor(
                        yf[:], u[h][:, cs:cs + TCH], Dv_s[:, h:h + 1],
                        pY[h][0:96, :], op0=ALU.mult, op1=ALU.add)
                    szl = tlp.tile([96, TCH], F32, name='szl', tag='szl',
                                   bufs=2)
                    nc.scalar.activation(szl[:], z[h][:, cs:cs + TCH],
                                         AF.Sigmoid)
                    szm = tlp.tile([96, TCH], F32, name='szm', tag='szm',
                                   bufs=2)
                    nc.vector.tensor_tensor(szm[:], szl[:],
                                            z[h][:, cs:cs + TCH], op=ALU.mult)
                    yf2 = tlp.tile([96, TCH], F32, name='yf2', tag='yf2',
                                   bufs=2)
                    nc.vector.tensor_tensor(yf2[:], yf[:], szm[:], op=ALU.mult)
                    nc.tensor.matmul(pZ[:], G_s[h][:], yf2[:],
                                     start=(h == 0), stop=(h == 1))
                zc = tlp.tile([96, TCH], F32, name='zc', tag='zc', bufs=2)
                nc.scalar.copy(zc[:], pZ[:])
                nc.sync.dma_start(zstage[:, cs:cs + TCH], zc[:])

        # ---- Phase C: mask-blend un-reversal, fuse-sum, batchnorm ----
        with tc.tile_pool(name="ep", bufs=1) as ep:
            for c in range(NCH):
                cs = c * TCH
                zn = ep.tile([96, TCH], F32, name='zn', tag='zn', bufs=2)
                nc.sync.dma_start(zn[:], zstage[:, cs:cs + TCH])
                zr = ep.tile([96, TCH], F32, name='zr', tag='zr', bufs=2)
                nc.sync.dma_start(zr[:], zstage[:, ::-1][:, cs:cs + TCH])
                za = ep.tile([96, TCH], F32, name='za', tag='za', bufs=2)
                nc.vector.tensor_scalar(za[:], zn[:], m0_s[:, 0:1], None,
                                        op0=ALU.mult)
                zb = ep.tile([96, TCH], F32, name='zb', tag='zb', bufs=2)
                nc.vector.scalar_tensor_tensor(zb[:], zr[:], m1_s[:, 0:1],
                                               za[:], op0=ALU.mult,
                                               op1=ALU.add)
                nc.sync.dma_start(zin[:, cs:cs + TCH], zb[:])
            nc.gpsimd.collective_compute(
                "AllReduce", ALU.add,
                replica_groups=[[0, 4], [1, 5], [2, 6], [3, 7]],
                ins=[zin.opt()], outs=[zred.opt()])
            yp = ep.tile([96, L], F32, name='yp', tag='yp')
            nc.sync.dma_start(yp[:], zred[:])
            s1p = ep.tile([96, NCH], F32, name='s1p', tag='s1p')
            s2p = ep.tile([96, NCH], F32, name='s2p', tag='s2p')
            for c in range(NCH):
                cs = c * TCH
                nc.vector.tensor_reduce(s1p[:, c:c + 1], yp[:, cs:cs + TCH],
                                        op=ALU.add, axis=mybir.AxisListType.X)
                sq = ep.tile([96, TCH], F32, name='sq', tag='sq', bufs=2)
                nc.gpsimd.tensor_tensor(sq[:], yp[:, cs:cs + TCH],
                                        yp[:, cs:cs + TCH], op=ALU.mult)
                nc.vector.tensor_reduce(s2p[:, c:c + 1], sq[:],
                                        op=ALU.add, axis=mybir.AxisListType.X)
            stats = ep.tile([96, 2], F32, name='stats', tag='stats')
            nc.vector.tensor_reduce(stats[:, 0:1], s1p[:], op=ALU.add,
                                    axis=mybir.AxisListType.X)
            nc.vector.tensor_reduce(stats[:, 1:2], s2p[:], op=ALU.add,
                                    axis=mybir.AxisListType.X)
            nc.sync.dma_start(stin[:], stats[:])
            nc.gpsimd.collective_compute(
                "AllReduce", ALU.add,
                replica_groups=[list(range(NCORES))],
                ins=[stin.opt()], outs=[stred.opt()])
            ss = ep.tile([96, 2], F32, name='ss', tag='ss')
            nc.sync.dma_start(ss[:], stred[:])
            sv = ep.tile([96, 8], F32, name='sv', tag='sv')
            # sv cols: 0 m_y, 1 ex2, 2 m_y^2, 3 var, 4 sd, 5 rinv
            nc.vector.tensor_scalar(sv[:, 0:2], ss[:], 1.0 / BN_CNT, None,
                                    op0=ALU.mult)
            nc.vector.tensor_tensor(sv[:, 2:3], sv[:, 0:1], sv[:, 0:1],
                                    op=ALU.mult)
            nc.vector.tensor_tensor(sv[:, 3:4], sv[:, 1:2], sv[:, 2:3],
                                    op=ALU.subtract)
            nc.vector.tensor_scalar(sv[:, 6:7], sv[:, 3:4], 1e-5, None,
                                    op0=ALU.add)
            nc.scalar.activation(sv[:, 4:5], sv[:, 6:7], AF.Sqrt)
            nc.vector.reciprocal(sv[:, 5:6], sv[:, 4:5])
            bnp = ep.tile([96, 3], F32, name='bnp', tag='bnp')
            # bnp cols: 0 scale, 1 shift, 2 scratch
            nc.vector.tensor_tensor(bnp[:, 0:1], bng_s[:], sv[:, 5:6],
                                    op=ALU.mult)
            # shift = bnb - (m_y + fuseb) * scale ; out = relu(yp*scale+shift)
            nc.vector.tensor_tensor(bnp[:, 2:3], sv[:, 0:1], fuseb_s[:],
                                    op=ALU.add)
            nc.vector.tensor_tensor(bnp[:, 2:3], bnp[:, 2:3], bnp[:, 0:1],
                                    op=ALU.mult)
            nc.vector.tensor_tensor(bnp[:, 1:2], bnb_s[:], bnp[:, 2:3],
                                    op=ALU.subtract)
            # fold fuseb back in: out = relu((yp+fuseb)*s + bnb - mean_t*s)
            nc.vector.tensor_tensor(bnp[:, 2:3], fuseb_s[:], bnp[:, 0:1],
                                    op=ALU.mult)
            nc.vector.tensor_tensor(bnp[:, 1:2], bnp[:, 1:2], bnp[:, 2:3],
                                    op=ALU.add)
            for c in range(NCH):
                cs = c * TCH
                oc = ep.tile([96, TCH], F32, name='oc', tag='oc', bufs=2)
                nc.scalar.activation(oc[:], yp[:, cs:cs + TCH], AF.Relu,
                                     bias=bnp[:, 1:2], scale=bnp[:, 0:1])
                nc.sync.dma_start(out[:, cs:cs + TCH], oc[:])

    nc.compile()
    return nc


def host_inputs(inputs):
    """Build the 8 per-core input dicts from the full problem inputs."""
    x = np.asarray(inputs["x"], np.float32)           # (4, 96, 64, 64)
    jj = np.arange(128)
    e8b = np.zeros((128, 512), np.float32)
    selb = np.zeros((128, 128), np.float32)
    for v in range(4):
        e8b[:, 128 * v:128 * (v + 1)] = (
            (jj % 32)[:, None] == (8 * v + jj // 16)[None, :])
        selb[:, 32 * v:32 * (v + 1)] = (
            (8 * v + jj // 16)[:, None] == np.arange(32)[None, :])
    e16d = np.zeros((48, 128), np.float32)
    e16 = (np.arange(NST)[:, None] == (jj % 16)[None, :]).astype(np.float32)
    e16d[0:16] = e16
    e16d[32:48] = e16

    in_maps = []
    for core in range(NCORES):
        d = "f" if core < 4 else "b"
        b = core % 4
        g = lambda n: np.asarray(inputs[f"{d}_{n}"], np.float32)
        xt = np.ascontiguousarray(x[b].reshape(DIM, L))
        if d == "b":
            xt = np.ascontiguousarray(xt[:, ::-1])
        A = -np.exp(g("A_log"))                        # (192, 16)
        asc = np.stack([A[8 * t + jj // 16, jj % 16] for t in range(NT)],
                       axis=1)
        fuse_half = np.asarray(inputs["fuse_W"], np.float32)[
            :, (0 if d == "f" else DIM):(DIM if d == "f" else 2 * DIM)]
        G = g("out_W") @ fuse_half.T                   # (192, 96)
        xpw = g("xproj_W")                             # (192, 38)
        xpp = np.zeros((DI, XP_M), np.float32)
        xpp[:, XB:XB + NST] = xpw[:, DTR:DTR + NST]    # B
        xpp[:, XC_:XC_ + NST] = xpw[:, DTR + NST:]     # C
        xpp[:, XD:XD + DTR] = xpw[:, 0:DTR]            # dt
        cw = g("conv_w")                               # (192, 4)
        m = np.ones((DIM, 1), np.float32)
        zo = np.zeros((DIM, 1), np.float32)
        in_maps.append({
            "xT": xt,
            "inW": g("in_W"),
            "convw": np.concatenate([cw[0:96, :], cw[96:192, :]], axis=1),
            "convb": np.stack([g("conv_b")[0:96], g("conv_b")[96:192]], 1),
            "xpWp": xpp,
            "dtW": g("dt_W"),
            "dtb": np.stack([g("dt_b")[0:96], g("dt_b")[96:192]], 1),
            "Dv": np.stack([g("D")[0:96], g("D")[96:192]], 1),
            "Asc": np.ascontiguousarray(asc),
            "E8B": e8b, "E16D": e16d, "SELB": selb,
            "Gw": np.ascontiguousarray(G),
            "m0": m if d == "f" else zo,
            "m1": zo if d == "f" else m,
            "fuseb": np.asarray(inputs["fuse_b"], np.float32).reshape(DIM, 1),
            "bng": np.asarray(inputs["bn_g"], np.float32).reshape(DIM, 1),
            "bnb": np.asarray(inputs["bn_b"], np.float32).reshape(DIM, 1),
        })
    return in_maps


_NC_CACHE = {}


def kernel(**inputs) -> np.ndarray:
    if "nc" not in _NC_CACHE:
        _NC_CACHE["nc"] = build_program()
    nc = _NC_CACHE["nc"]
    in_maps = host_inputs(inputs)
    res = run_bass_kernel_spmd(nc, in_maps, list(range(NCORES)))
    outs = [np.asarray(res.results[b]["out"], np.float32).reshape(DIM, HH, WW)
            for b in range(BSZ)]
    return np.stack(outs, axis=0)
